# revision 17
# baseline (speedup 1.0000x reference)
"""Trainium2 Bass kernel for nn_AttentionDecoder.

Computation (per batch b):
  s_t       = concat(h_dec, c_dec)                      # (D,)
  enc_feat  = E @ Wh^T                                  # (S, D)
  dec_feat  = s_t @ Ws^T + Ws_b                         # (D,)
  score     = tanh(enc_feat + dec_feat + cov*wc) @ v    # (S,)
  w         = exp(score) * mask
  attn      = w / sum(w)
  context   = attn @ E                                  # (D,)
  cov_new   = cov + attn

Sharding: data-parallel over batch, 4 batch elements per core, 8 cores.
Layout: s = 32*p + i (p = partition 0..127, i = s-tile 0..31) so that
[128, 32]-shaped per-b score/attn/coverage tensors map to contiguous
128B-per-partition DMAs, and E tiles load as 8KB-contiguous-per-partition
mega tiles (4 s-tiles per DMA).

E is loaded once from HBM with an SWDGE cast-DMA (f32 -> bf16), giving
bf16 operands for both the feat matmul (via PE transposes, 1 cyc/row) and
the context matmul, with fp32 PSUM accumulation.  dec_feat + cov (x) wc
enter the feat PSUM group as a K=2 rank-2 matmul ([ones; cov] x [dec; wc]).
Score / softmax / renorm / coverage math stays fp32.
"""

import os
import sys

import numpy as np

sys.path.insert(0, "/opt/trn_rl_repo")

B, S, H = 32, 4096, 256
D = 2 * H  # 512
NCORES = 8
BPC = B // NCORES  # 4
NMEGA = 8          # mega-tiles per batch element (4 s-tiles each)
NTILE = 32         # s-tiles per batch element (128 rows each)

_cache = {}


def _build(skip=()):
    import concourse.bass as bass
    from concourse import bacc
    import concourse.mybir as mybir
    import concourse.tile as tile
    from concourse.masks import make_identity

    f32 = mybir.dt.float32
    bf16 = mybir.dt.bfloat16
    AF = mybir.ActivationFunctionType
    OP = mybir.AluOpType
    AX = mybir.AxisListType

    nc = bacc.Bacc("TRN2", target_bir_lowering=False, debug=False)

    # ---- DRAM I/O (per-core shapes) ----
    enc = nc.dram_tensor("enc", [BPC, S, D], f32, kind="ExternalInput")
    cov = nc.dram_tensor("cov", [BPC, S], f32, kind="ExternalInput")
    msk = nc.dram_tensor("msk", [BPC, S], f32, kind="ExternalInput")
    stT = nc.dram_tensor("stT", [128, 4 * BPC], bf16, kind="ExternalInput")  # col=4c+b
    whT = nc.dram_tensor("whT", [128, 4 * D], bf16, kind="ExternalInput")    # chunk c at cols 512c
    wsT = nc.dram_tensor("wsT", [128, 4 * D], bf16, kind="ExternalInput")
    wsb = nc.dram_tensor("wsb", [1, D], bf16, kind="ExternalInput")
    wc4 = nc.dram_tensor("wc4", [1, BPC * D], bf16, kind="ExternalInput")    # wc tiled x4
    agc = nc.dram_tensor("agc", [BPC, 2, S], bf16, kind="ExternalInput")     # [ones; cov] rows
    vrp = nc.dram_tensor("vrp", [128, D], bf16, kind="ExternalInput")        # v replicated

    ctx_o = nc.dram_tensor("ctx_o", [BPC, D], f32, kind="ExternalOutput")
    att_o = nc.dram_tensor("att_o", [BPC, S], f32, kind="ExternalOutput")
    cvn_o = nc.dram_tensor("cvn_o", [BPC, S], f32, kind="ExternalOutput")

    # DRAM views
    # E mega tile j of batch b: rows s = 32p + 4j + t  -> [128, 4*512]
    enc_r = enc.rearrange("b (p j t) d -> b j p (t d)", p=128, j=NMEGA, t=4)
    cov_pi = cov.rearrange("b (p i) -> b p i", p=128)     # [b, 128, 32]
    msk_pi = msk.rearrange("b (p i) -> b p i", p=128)
    att_pi = att_o.rearrange("b (p i) -> b p i", p=128)
    cvn_pi = cvn_o.rearrange("b (p i) -> b p i", p=128)

    with tile.TileContext(nc) as tc:
        import contextlib

        ctxs = contextlib.ExitStack()
        with ctxs:
            singles = ctxs.enter_context(tc.tile_pool(name="singles", bufs=1))
            e_pool = ctxs.enter_context(tc.tile_pool(name="e", bufs=33))
            et_pool = ctxs.enter_context(tc.tile_pool(name="et", bufs=4))
            t_pool = ctxs.enter_context(tc.tile_pool(name="t", bufs=4))
            scr_pool = ctxs.enter_context(tc.tile_pool(name="scr", bufs=3))
            sc_pool = ctxs.enter_context(tc.tile_pool(name="sc", bufs=2))
            aug_pool = ctxs.enter_context(tc.tile_pool(name="aug", bufs=2))
            sm_pool = ctxs.enter_context(tc.tile_pool(name="sm", bufs=4))
            ps_feat = ctxs.enter_context(tc.tile_pool(name="psf", bufs=2, space="PSUM"))
            ps_et_a = ctxs.enter_context(tc.tile_pool(name="psea", bufs=2, space="PSUM"))
            ps_et_d = ctxs.enter_context(tc.tile_pool(name="psed", bufs=2, space="PSUM"))
            ps_ctx = ctxs.enter_context(tc.tile_pool(name="psc", bufs=1, space="PSUM"))
            ps_misc = ctxs.enter_context(tc.tile_pool(name="psm", bufs=1, space="PSUM"))

            # ---- constants ----
            whT_sb = singles.tile([128, 4 * D], bf16)
            nc.sync.dma_start(out=whT_sb, in_=whT[:, :])
            wsT_sb = singles.tile([128, 4 * D], bf16)
            nc.sync.dma_start(out=wsT_sb, in_=wsT[:, :])
            stT_sb = singles.tile([128, 4 * BPC], bf16)
            nc.sync.dma_start(out=stT_sb, in_=stT[:, :])
            wsb_sb = singles.tile([1, D], bf16)
            nc.sync.dma_start(out=wsb_sb, in_=wsb[:, :])
            vrp_sb = singles.tile([128, D], bf16)
            nc.sync.dma_start(out=vrp_sb, in_=vrp[:, :])
            augw = singles.tile([2, BPC * D], bf16)       # row0 = dec_b, row1 = wc
            nc.sync.dma_start(out=augw[1:2, :], in_=wc4[:, :])
            ident = singles.tile([128, 128], bf16)
            make_identity(nc, ident)
            ones1 = singles.tile([1, 1], bf16)
            nc.gpsimd.memset(ones1, 1.0)
            o128 = singles.tile([128, 1], f32)
            nc.gpsimd.memset(o128, 1.0)
            orow = singles.tile([1, 128], f32)
            nc.gpsimd.memset(orow, 1.0)

            all_megas = [[] for _ in range(BPC)]
            all_attnb = [None] * BPC
            for b in range(BPC):
                # ---- per-b setup ----
                # dec_feat_b = s_t[b] @ Ws^T + Ws_b   -> augw[0, 512b:512b+512]
                pd = ps_misc.tile([1, D], f32, tag="m")
                if "dec" in skip:
                    nc.vector.memset(pd, 0.0)
                else:
                  for c in range(4):
                    nc.tensor.matmul(
                        pd,
                        lhsT=stT_sb[:, 4 * c + b : 4 * c + b + 1],
                        rhs=wsT_sb[:, D * c : D * (c + 1)],
                        start=(c == 0),
                        stop=False,
                    )
                  nc.tensor.matmul(
                      pd, lhsT=ones1, rhs=wsb_sb, start=False, stop=True
                  )
                nc.scalar.copy(out=augw[0:1, D * b : D * (b + 1)], in_=pd)

                aug = aug_pool.tile([2, S], bf16)         # row0 = ones, row1 = cov
                nc.sync.dma_start(out=aug, in_=agc[b])
                aug_r = aug.rearrange("k (p i) -> k p i", p=128)

                covp = sm_pool.tile([128, NTILE], f32)
                nc.sync.dma_start(out=covp, in_=cov_pi[b])
                mskp = sm_pool.tile([128, NTILE], f32)
                nc.sync.dma_start(out=mskp, in_=msk_pi[b])

                score = sc_pool.tile([128, NTILE], f32)
                e_megas = all_megas[b]

                for j in range(NMEGA):
                    em = e_pool.tile([128, 4 * D], bf16)
                    nc.gpsimd.dma_start(out=em, in_=enc_r[b, j])  # casts f32->bf16
                    e_megas.append(em)

                    # transpose the 4 d-chunks of these 4 s-tiles
                    et = et_pool.tile([128, 4 * D], bf16)  # chunk c at cols 512c, col=128t+p
                    for c2 in range(2):
                        pool = ps_et_a if c2 == 0 else ps_et_d
                        pe_t = pool.tile([128, 2 * D], bf16)
                        for cc in range(2):
                            c = 2 * c2 + cc
                            for t in range(4):
                                nc.tensor.transpose(
                                    pe_t[:, D * cc + 128 * t : D * cc + 128 * (t + 1)],
                                    in_=em[:, D * t + 128 * c : D * t + 128 * (c + 1)],
                                    identity=ident,
                                )
                        cb = nc.scalar.copy if c2 == 0 else nc.vector.tensor_copy
                        cb(out=et[:, 2 * D * c2 : 2 * D * (c2 + 1)], in_=pe_t)

                    for t in range(4):
                        i = 4 * j + t
                        pf = ps_feat.tile([128, D], f32)
                        if "aug" not in skip:
                            # rank-2 update: ones x dec_b + cov x wc
                            nc.tensor.matmul(
                                pf,
                                lhsT=aug_r[:, :, i],
                                rhs=augw[:, D * b : D * (b + 1)],
                                start=True,
                                stop=False,
                            )
                        for c in range(4):
                            nc.tensor.matmul(
                                pf,
                                lhsT=et[:, D * c + 128 * t : D * c + 128 * (t + 1)],
                                rhs=whT_sb[:, D * c : D * (c + 1)],
                                start=("aug" in skip and c == 0),
                                stop=(c == 3),
                            )
                        tt = t_pool.tile([128, D], bf16)
                        if "tanh" in skip:
                            nc.scalar.copy(out=tt, in_=pf)
                        else:
                            nc.scalar.activation(out=tt, in_=pf, func=AF.Tanh)
                        if "ttr" in skip:
                            nc.vector.tensor_reduce(
                                score[:, i : i + 1], tt, axis=AX.X, op=OP.add
                            )
                        else:
                            scr = scr_pool.tile([128, D], bf16)
                            nc.vector.affine_mul_reduce(
                                out=scr,
                                accum_out=score[:, i : i + 1],
                                in0=tt,
                                in1=vrp_sb,
                                scale=1.0,
                                bias=0.0,
                            )

                # ---- softmax (no max-sub: |score| <= sum|v| ~ 8) ----
                if "sm" in skip:
                    wm = score
                else:
                    w = sm_pool.tile([128, NTILE], f32)
                    nc.scalar.activation(out=w, in_=score, func=AF.Exp)
                    wm = sm_pool.tile([128, NTILE], f32)
                    nc.vector.tensor_mul(wm, w, mskp)
                zc = sm_pool.tile([128, 1], f32)
                nc.vector.tensor_reduce(zc, wm, axis=AX.X, op=OP.add)
                if "zmm" in skip:
                    rz = sm_pool.tile([128, 1], f32)
                    nc.vector.reciprocal(rz, zc)
                else:
                    pz = ps_misc.tile([1, 1], f32, tag="m")
                    nc.tensor.matmul(pz, lhsT=o128, rhs=zc, start=True, stop=True)
                    zs = sm_pool.tile([1, 1], f32)
                    nc.scalar.copy(out=zs, in_=pz)
                    pzr = ps_misc.tile([128, 1], f32, tag="m")
                    nc.tensor.matmul(pzr, lhsT=orow, rhs=zs, start=True, stop=True)
                    rz = sm_pool.tile([128, 1], f32)
                    nc.vector.reciprocal(rz, pzr)
                attn = sm_pool.tile([128, NTILE], f32)
                nc.vector.tensor_scalar_mul(attn, in0=wm, scalar1=rz)
                attnb = sm_pool.tile([128, NTILE], bf16)
                nc.vector.tensor_copy(out=attnb, in_=attn)
                all_attnb[b] = attnb

                cvn = sm_pool.tile([128, NTILE], f32)
                nc.vector.tensor_add(cvn, attn, covp)
                nc.sync.dma_start(out=att_pi[b], in_=attn)
                nc.sync.dma_start(out=cvn_pi[b], in_=cvn)

            # ---- context = attn @ E: all 4 b concurrent via PE col-tiling ----
            pc = ps_ctx.tile([128, D], f32)
            if "ctx" in skip:
                nc.vector.memset(pc, 0.0)
            for j in ([] if "ctx" in skip else range(NMEGA)):
                for t in range(4):
                    i = 4 * j + t
                    for b in range(BPC):
                        nc.tensor.matmul(
                            pc[32 * b : 32 * b + 1, :],
                            lhsT=all_attnb[b][:, i : i + 1],
                            rhs=all_megas[b][j][:, D * t : D * (t + 1)],
                            start=(i == 0),
                            stop=(i == NTILE - 1),
                            tile_position=(0, 32 * b),
                            skip_group_check=True,
                        )
            for b in range(BPC):
                cxs = sm_pool.tile([1, D], f32)
                nc.scalar.copy(out=cxs, in_=pc[32 * b : 32 * b + 1, :])
                nc.sync.dma_start(out=ctx_o[b : b + 1, :], in_=cxs)

    nc.finalize()
    return nc


def _prep_inputs(encoder_output, h_dec, c_dec, x_padding_masks, coverage_vector,
                 Wh_w, Ws_w, Ws_b, wc_w, v_w):
    """Build per-core in_maps (host-side layout prep; weights are tiny)."""
    import ml_dtypes

    bf = ml_dtypes.bfloat16
    enc = np.ascontiguousarray(np.asarray(encoder_output, dtype=np.float32))
    cov = np.ascontiguousarray(np.asarray(coverage_vector, dtype=np.float32))
    msk = np.ascontiguousarray(np.asarray(x_padding_masks, dtype=np.float32))
    s_t = np.concatenate(
        [np.asarray(h_dec, np.float32)[0], np.asarray(c_dec, np.float32)[0]], axis=1
    )  # (B, D)

    def chunked(wT):  # (D, D) [d, e] -> [128, 4*D] chunk c at cols 512c
        return np.ascontiguousarray(
            wT.reshape(4, 128, D).transpose(1, 0, 2).reshape(128, 4 * D).astype(bf)
        )

    whT = chunked(np.asarray(Wh_w, np.float32).T.copy())
    wsT = chunked(np.asarray(Ws_w, np.float32).T.copy())
    wsb = np.asarray(Ws_b, np.float32).reshape(1, D).astype(bf)
    wc4 = np.tile(np.asarray(wc_w, np.float32).reshape(1, D), (1, BPC)).astype(bf)
    vrp = np.tile(np.asarray(v_w, np.float32).reshape(1, D), (128, 1)).astype(bf)

    agc_full = np.ones((B, 2, S), dtype=bf)
    agc_full[:, 1, :] = cov.astype(bf)

    in_maps = []
    for k in range(NCORES):
        bs = slice(k * BPC, (k + 1) * BPC)
        stT = np.ascontiguousarray(
            s_t[bs].T.reshape(4, 128, BPC).transpose(1, 0, 2).reshape(128, 4 * BPC)
        ).astype(bf)  # col = 4c+b
        in_maps.append(
            {
                "enc": np.ascontiguousarray(enc[bs]),
                "cov": np.ascontiguousarray(cov[bs]),
                "msk": np.ascontiguousarray(msk[bs]),
                "stT": stT,
                "whT": whT,
                "wsT": wsT,
                "wsb": wsb,
                "wc4": wc4,
                "agc": np.ascontiguousarray(agc_full[bs]),
                "vrp": vrp,
            }
        )
    return in_maps


def _run(inputs, trace=False, tmpdir=None, skip=()):
    from concourse import bass_utils

    key = ("nc",) + tuple(sorted(skip))
    if key not in _cache:
        _cache[key] = _build(skip)
    nc = _cache[key]
    in_maps = _prep_inputs(**inputs)
    res = bass_utils.run_bass_kernel_spmd(
        nc, in_maps, core_ids=list(range(NCORES)), trace=trace, tmpdir=tmpdir
    )
    ctx = np.concatenate([om["ctx_o"] for om in res.results], axis=0)
    att = np.concatenate([om["att_o"] for om in res.results], axis=0)
    cvn = np.concatenate([om["cvn_o"] for om in res.results], axis=0)
    return (ctx, att, cvn), res


def kernel(**inputs):
    (ctx, att, cvn), _ = _run(inputs, trace=False)
    return ctx, att, cvn
